# revision 1
# baseline (speedup 1.0000x reference)
"""Causal self-attention (GQA + RMS-norm + partial RoPE) Trainium2 kernel.

Full inputs in, full output out. Sharding: 8 cores = batch(4) x head-half(2).
Each core computes its batch's QKV for 8 q-heads / 2 kv-heads entirely in
transposed layouts (head_dim on partitions), does causal attention with a
no-max softmax (scores bounded by sqrt(hd) after RMS norm), and a row-sharded
output projection; the host sums the two half partials per batch.

All large matmuls run in float32r (TF32-like, full PE rate at N=512).
Single long-lived PSUM pool with 8 rotating bank tags (t0..t7) — no
pool-scope barriers anywhere in the hot path.
"""
import numpy as np

import concourse.bacc as bacc
import concourse.mybir as mybir
from concourse.tile import TileContext
from concourse.bass_utils import run_bass_kernel_spmd

F32 = mybir.dt.float32
F32R = mybir.dt.float32r
AF = mybir.ActivationFunctionType

B, S, D = 4, 2048, 2048
H, KV, HD = 16, 4, 128
ROPE, HALF_ROPE = 64, 32
EPS = 1.1920929e-07
N_CORES = 8
NDC = D // 128          # 16 contraction chunks
NQC = S // 512          # 4 query chunks of 512
LH = 8                  # local q heads per core
LKV = 2                 # local kv heads per core

_cached_program = None
_last_in_maps = None


def _build_program():
    nc = bacc.Bacc("TRN2")
    # eps const AP for activation bias
    t = nc.alloc_sbuf_tensor("const-f32-eps", [128, 1], F32)
    nc.gpsimd.memset(t.ap(), EPS)
    nc.const_aps.aps[(F32, EPS)] = t.ap()
    nc.all_engine_barrier()

    xT = nc.declare_dram_parameter("xT", [D, S], F32R, isOutput=False)
    wqT = nc.declare_dram_parameter("wqT", [D, LH * HD], F32R, isOutput=False)
    wkT = nc.declare_dram_parameter("wkT", [D, LKV * HD], F32R, isOutput=False)
    wvT = nc.declare_dram_parameter("wvT", [D, LKV * HD], F32R, isOutput=False)
    wpT = nc.declare_dram_parameter("wpT", [LH * HD, D], F32R, isOutput=False)
    cosT = nc.declare_dram_parameter("cosT", [HALF_ROPE, S], F32, isOutput=False)
    sinT = nc.declare_dram_parameter("sinT", [HALF_ROPE, S], F32, isOutput=False)
    o128d = nc.declare_dram_parameter("ones128", [128, 1], F32R, isOutput=False)
    obd = nc.declare_dram_parameter("onesb", [1, 128], F32R, isOutput=False)
    gaind = nc.declare_dram_parameter("gains", [128, LH], F32, isOutput=False)
    out = nc.declare_dram_parameter("out", [S, D], F32, isOutput=True)

    with TileContext(nc) as tc:
        with (
            tc.tile_pool(name="cp", bufs=1) as cp,
            tc.tile_pool(name="xap", bufs=1) as xap,
            tc.tile_pool(name="wqp", bufs=2) as wqp,
            tc.tile_pool(name="wpp", bufs=2) as wpp,
            tc.tile_pool(name="stgp", bufs=2) as stgp,
            tc.tile_pool(name="scr", bufs=2) as scr,
            tc.tile_pool(name="exp3", bufs=3) as exp3,
            tc.tile_pool(name="qnp", bufs=1) as qnp,
            tc.tile_pool(name="pu", bufs=1, space="PSUM") as pu,
        ):
            wk_t = cp.tile([128, NDC, LKV * HD], F32R, tag="wk")
            nc.sync.dma_start(out=wk_t[:],
                              in_=wkT.rearrange("(c p) e -> p c e", p=128))
            wv_t = cp.tile([128, NDC, LKV * HD], F32R, tag="wv")
            nc.sync.dma_start(out=wv_t[:],
                              in_=wvT.rearrange("(c p) e -> p c e", p=128))
            cos_t = cp.tile([HALF_ROPE, S], F32, tag="cos")
            nc.sync.dma_start(out=cos_t[:], in_=cosT[:])
            sin_t = cp.tile([HALF_ROPE, S], F32, tag="sin")
            nc.sync.dma_start(out=sin_t[:], in_=sinT[:])
            o128 = cp.tile([128, 1], F32R, tag="o128")
            nc.sync.dma_start(out=o128[:], in_=o128d[:])
            ob = cp.tile([1, 128], F32R, tag="ob")
            nc.sync.dma_start(out=ob[:], in_=obd[:])
            gains = cp.tile([128, LH], F32, tag="gains")
            nc.sync.dma_start(out=gains[:], in_=gaind[:])
            kn_t = cp.tile([128, LKV, S], F32R, tag="kn")
            v_t = cp.tile([128, S // 128, LKV * HD], F32R, tag="v")

            def bank(i, shape=(128, 512), dt=F32, nm=None):
                return pu.tile(list(shape), dt, tag=f"t{i}",
                               name=nm or f"pt{i}")

            def load_x(pos0):
                """x chunk [128, NDC, 512] as 16 per-dc DMAs (compute on
                chunk dc can start as soon as its DMA lands)."""
                xa = xap.tile([128, NDC, 512], F32R, tag="xa", name="xa")
                for dc in range(NDC):
                    nc.sync.dma_start(
                        out=xa[:, dc],
                        in_=xT[dc * 128:(dc + 1) * 128, pos0:pos0 + 512])
                return xa

            def norm_rope(raw, out_ap, pos0):
                """RMS-norm + partial RoPE: transposed raw [128,512] (PSUM)
                -> out_ap ([128,512] f32r). pos0 selects cos/sin columns."""
                cs = slice(pos0, pos0 + 512)
                sq = scr.tile([128, 512], F32R, tag="sq", name="sq")
                nc.scalar.activation(sq[:], raw[:], AF.Square)
                ssq = bank(6, (1, 512), nm="ssq")
                nc.tensor.matmul(ssq[:], o128[:], sq[:], start=True, stop=True)
                # r = rsqrt(ssq/128 + eps) = sqrt(1/(ssq/128 + eps))
                u = scr.tile([1, 512], F32, tag="u", name="u")
                nc.scalar.activation(u[:], ssq[:], AF.Copy,
                                     scale=1.0 / HD, bias=EPS)
                uin = scr.tile([1, 512], F32, tag="uin", name="uin")
                nc.vector.reciprocal_approx_fast(out=uin[:], in_=u[:])
                r = scr.tile([1, 512], F32R, tag="rr", name="rr")
                nc.scalar.activation(r[:], uin[:], AF.Sqrt)
                Rp = bank(7, nm="Rp")
                nc.tensor.matmul(Rp[:], ob[:], r[:], start=True, stop=True)
                # cr/sr read R straight from PSUM (1 psum operand is legal);
                # only the rope pass-through rows need an SBUF copy of R
                Rsb = scr.tile([128, 512], F32, tag="Rsb", name="Rsb")
                nc.scalar.copy(Rsb[ROPE:128, :], Rp[ROPE:128, :])
                cr = scr.tile([HALF_ROPE, 512], F32, tag="cr", name="cr")
                sr = scr.tile([HALF_ROPE, 512], F32, tag="sr", name="sr")
                nc.vector.tensor_mul(cr[:], cos_t[:, cs], Rp[0:HALF_ROPE, :])
                nc.vector.tensor_mul(sr[:], sin_t[:, cs], Rp[0:HALF_ROPE, :])
                tmp = scr.tile([ROPE, 512], F32, tag="tmp", name="tmp")
                h1, h2 = slice(0, HALF_ROPE), slice(HALF_ROPE, ROPE)
                nc.vector.tensor_mul(out_ap[h1, :], raw[h1, :], cr[:])
                nc.vector.tensor_mul(tmp[h1, :], raw[h2, :], sr[:])
                nc.vector.tensor_add(out_ap[h1, :], out_ap[h1, :], tmp[h1, :])
                nc.vector.tensor_mul(out_ap[h2, :], raw[h2, :], cr[:])
                nc.vector.tensor_mul(tmp[h2, :], raw[h1, :], sr[:])
                nc.vector.tensor_sub(out_ap[h2, :], out_ap[h2, :], tmp[h2, :])
                nc.vector.tensor_mul(out_ap[ROPE:128, :], raw[ROPE:128, :],
                                     Rsb[ROPE:128, :])

            # ---------------- Phase A: kT (norm+rope) and v ----------------
            for sc in range(NQC):
                xa = load_x(sc * 512)
                for g in range(LKV):
                    kacc = bank(g, nm=f"kacc{g}")
                    for dc in range(NDC):
                        nc.tensor.matmul(
                            kacc[:], wk_t[:, dc, g * HD:(g + 1) * HD],
                            xa[:, dc], start=(dc == 0), stop=(dc == NDC - 1))
                    norm_rope(kacc, kn_t[:, g, sc * 512:(sc + 1) * 512],
                              sc * 512)
                for st4 in range(4):
                    st = sc * 4 + st4
                    vacc = bank(2 + st4 % 2, (128, LKV * HD), nm=f"vacc{st4}")
                    for dc in range(NDC):
                        nc.tensor.matmul(
                            vacc[:], xa[:, dc, st4 * 128:(st4 + 1) * 128],
                            wv_t[:, dc], start=(dc == 0), stop=(dc == NDC - 1))
                    nc.vector.tensor_copy(v_t[:, st], vacc[:])

            # ------------- Phase C: per query chunk q/attn/proj -------------
            for qc in range(NQC):
                pos0 = qc * 512
                n_kt = (qc + 1) * 4

                # -- q projection (two groups of 4 heads) + norm + rope --
                xa = load_x(pos0)
                qn = {}
                for grp in range(2):
                    qraw = {}
                    for dc in range(NDC):
                        wqt = wqp.tile([128, 512], F32R, tag="wq", name="wq")
                        nc.sync.dma_start(
                            out=wqt[:],
                            in_=wqT[dc * 128:(dc + 1) * 128,
                                    grp * 512:(grp + 1) * 512])
                        for hh in range(4):
                            if dc == 0:
                                qraw[hh] = bank(hh, nm=f"qraw{hh}")
                            nc.tensor.matmul(
                                qraw[hh][:], wqt[:, hh * HD:(hh + 1) * HD],
                                xa[:, dc], start=(dc == 0),
                                stop=(dc == NDC - 1))
                    for hh in range(4):
                        h = grp * 4 + hh
                        qn[h] = qnp.tile([128, 512], F32R, tag=f"qn{h}",
                                         name=f"qn{h}")
                        norm_rope(qraw[hh], qn[h][:], pos0)

                # -- attention --
                yt_sb = {}
                for h in range(LH):
                    g = h // 4
                    yt_ps = bank(h % 2, nm=f"yt{h}")
                    l_ps = bank(2 + h % 2, (1, 512), nm=f"l{h}")
                    for kt in range(n_kt):
                        sc_ps = bank(4 + kt % 4, nm=f"sc{h}_{kt}")
                        nc.tensor.matmul(
                            sc_ps[:], kn_t[:, g, kt * 128:(kt + 1) * 128],
                            qn[h][:], start=True, stop=True)
                        ex = exp3.tile([128, 512], F32R, tag="ex", name="ex")
                        nc.scalar.activation(ex[:], sc_ps[:], AF.Exp,
                                             scale=gains[:, h:h + 1])
                        m = kt - qc * 4
                        if m >= 0:
                            # zero ex where key pos > query pos:
                            # keep iff n - p - 128*m >= 0
                            nc.gpsimd.affine_select(
                                out=ex[:], in_=ex[:],
                                compare_op=mybir.AluOpType.is_ge,
                                fill=0.0, base=-128 * m,
                                pattern=[[1, 512]], channel_multiplier=-1)
                        nc.tensor.matmul(
                            yt_ps[:], v_t[:, kt, g * HD:(g + 1) * HD], ex[:],
                            start=(kt == 0), stop=(kt == n_kt - 1))
                        nc.tensor.matmul(
                            l_ps[:], o128[:], ex[:], start=(kt == 0),
                            stop=(kt == n_kt - 1))
                    lf = scr.tile([1, 512], F32, tag="lf", name="lf")
                    nc.vector.reciprocal_approx_fast(out=lf[:], in_=l_ps[:])
                    linv = scr.tile([1, 512], F32R, tag="linv", name="linv")
                    nc.scalar.copy(linv[:], lf[:])
                    Li_ps = bank(4 + (n_kt + h) % 4, nm=f"Li{h}")
                    nc.tensor.matmul(Li_ps[:], ob[:], linv[:],
                                     start=True, stop=True)
                    Lsb = scr.tile([128, 512], F32, tag="Lsb", name="Lsb")
                    nc.vector.tensor_copy(Lsb[:], Li_ps[:])
                    yt_sb[h] = qnp.tile([128, 512], F32R, tag=f"yts{h}",
                                        name=f"yts{h}")
                    nc.vector.tensor_mul(yt_sb[h][:], yt_ps[:], Lsb[:])

                # -- output projection: out[s_q, j] += yT.T @ wpT --
                # 4 j-columns of 512; 4 psum accumulators (t4..t7) per jcol
                for jcol in range(4):
                    prs = {}
                    for h in range(LH):
                        wpt = wpp.tile([128, 512], F32R, tag="wp", name="wp")
                        nc.sync.dma_start(
                            out=wpt[:],
                            in_=wpT[h * 128:(h + 1) * 128,
                                    jcol * 512:(jcol + 1) * 512])
                        for st4 in range(4):
                            if h == 0:
                                prs[st4] = bank(4 + st4, nm=f"pr{jcol}{st4}")
                            nc.tensor.matmul(
                                prs[st4][:],
                                yt_sb[h][:, st4 * 128:(st4 + 1) * 128],
                                wpt[:], start=(h == 0), stop=(h == LH - 1))
                    for st4 in range(4):
                        stg = stgp.tile([128, 512], F32, tag="stg", name="stg")
                        nc.vector.tensor_copy(stg[:], prs[st4][:])
                        nc.sync.dma_start(
                            out=out[pos0 + st4 * 128:pos0 + (st4 + 1) * 128,
                                    jcol * 512:(jcol + 1) * 512],
                            in_=stg[:])
    nc.compile()
    return nc


def _rope_tables():
    inv = 1.0 / (10000.0 ** (np.arange(0, ROPE, 2, dtype=np.float64) / ROPE))
    fr = np.outer(np.arange(S, dtype=np.float64), inv)  # [S, 32]
    return (np.cos(fr).T.astype(np.float32).copy(),
            np.sin(fr).T.astype(np.float32).copy())


def kernel(x, Wq, Wk, Wv, Wproj, q_gain):
    global _cached_program, _last_in_maps
    x = np.ascontiguousarray(np.asarray(x, dtype=np.float32))
    Wq = np.asarray(Wq, dtype=np.float32)
    Wk = np.asarray(Wk, dtype=np.float32)
    Wv = np.asarray(Wv, dtype=np.float32)
    Wproj = np.asarray(Wproj, dtype=np.float32)
    q_gain = np.asarray(q_gain, dtype=np.float32)

    cosT, sinT = _rope_tables()
    ones128 = np.ones((128, 1), dtype=np.float32)
    onesb = np.ones((1, 128), dtype=np.float32)
    scale = 1.0 / np.sqrt(HD)

    in_maps = []
    for core in range(N_CORES):
        b, half = core // 2, core % 2
        g0 = half * LKV
        gains = np.repeat((q_gain[half * LH:(half + 1) * LH] * scale)
                          [None, :], 128, axis=0).astype(np.float32)
        in_maps.append({
            "xT": np.ascontiguousarray(x[b].T),
            "wqT": np.ascontiguousarray(
                Wq[half * LH * HD:(half + 1) * LH * HD, :].T),
            "wkT": np.ascontiguousarray(
                Wk[g0 * HD:(g0 + LKV) * HD, :].T),
            "wvT": np.ascontiguousarray(
                Wv[g0 * HD:(g0 + LKV) * HD, :].T),
            "wpT": np.ascontiguousarray(
                Wproj[:, half * LH * HD:(half + 1) * LH * HD].T),
            "cosT": cosT, "sinT": sinT,
            "ones128": ones128, "onesb": onesb, "gains": gains,
        })

    _last_in_maps = in_maps
    if _cached_program is None:
        _cached_program = _build_program()
    res = run_bass_kernel_spmd(_cached_program, in_maps, list(range(N_CORES)))

    out = np.empty((B, S, D), dtype=np.float32)
    for b in range(B):
        out[b] = res.results[2 * b]["out"] + res.results[2 * b + 1]["out"]
    return out



# revision 4
# speedup vs baseline: 1.5070x; 1.5070x over previous
"""Causal self-attention (GQA + RMS-norm + partial RoPE) Trainium2 kernel.

Full inputs in, full output out. Sharding: 8 cores = batch(4) x head-half(2).
Each core computes its batch's QKV for 8 q-heads / 2 kv-heads in transposed
layouts (head_dim on partitions), causal attention with a no-max softmax
(scores bounded after RMS norm), and a row-sharded output projection; the
host sums the two half partials per batch.

Perf structure (v2):
- Single ACT table set: rsqrt via exp(-0.5*ln(u)); all ACT funcs in
  natural_log_exp_and_others -> one ACT_TABLE_LOAD, no thrash.
- K is stored roped but UN-normalized; its per-key 1/rms lands in the
  score-exp's per-partition scale AP (rkT, computed via N=1 matmuls in
  transposed layout). Q's 1/rms row gets gain/sqrt(hd) folded in via the
  exp bias (host passes ln(gain*scale)), then is broadcast across
  partitions by GpSimd partition_broadcast (no PE broadcast matmuls).
- Softmax 1/l likewise via ln/exp + partition_broadcast.
- exp outputs / V / yt / Wproj in bf16 (2x DVE, halved DMA + gpsimd mask).
- x loaded as 16 per-dc slices in a 16-deep pool so consecutive qc
  iterations overlap; output projection runs st4-outer 8-matmul chains
  (1 PSUM bank each); qraw/sc share a 4-bank rotation, yt/l/ssq take the
  other 4. Goal: PE never idles >3us (HAM stays at 2.4GHz).
"""
import numpy as np
import ml_dtypes

import concourse.bacc as bacc
import concourse.mybir as mybir
from concourse.tile import TileContext
from concourse.bass_utils import run_bass_kernel_spmd

F32 = mybir.dt.float32
F32R = mybir.dt.float32r
BF16 = mybir.dt.bfloat16
AF = mybir.ActivationFunctionType

B, S, D = 4, 2048, 2048
H, KV, HD = 16, 4, 128
ROPE, HALF_ROPE = 64, 32
EPS = 1.1920929e-07
N_CORES = 8
NDC = D // 128          # 16 contraction chunks
NQC = S // 512          # 4 query chunks of 512
LH = 8                  # local q heads per core
LKV = 2                 # local kv heads per core

_cached_program = None
_last_in_maps = None


def _build_program():
    nc = bacc.Bacc("TRN2")
    # const APs for activation bias immediates
    t = nc.alloc_sbuf_tensor("const-f32-eps", [128, 1], F32)
    nc.gpsimd.memset(t.ap(), EPS)
    nc.const_aps.aps[(F32, EPS)] = t.ap()
    tz = nc.alloc_sbuf_tensor("const-f32-zero", [128, 1], F32)
    nc.gpsimd.memset(tz.ap(), 0.0)
    nc.const_aps.aps[(F32, 0.0)] = tz.ap()
    nc.all_engine_barrier()

    xT = nc.declare_dram_parameter("xT", [D, S], F32R, isOutput=False)
    wqT = nc.declare_dram_parameter("wqT", [D, LH * HD], F32R, isOutput=False)
    wkT = nc.declare_dram_parameter("wkT", [D, LKV * HD], F32R, isOutput=False)
    wvT = nc.declare_dram_parameter("wvT", [D, LKV * HD], F32R, isOutput=False)
    wpB = nc.declare_dram_parameter("wpB", [LH * HD, D], BF16, isOutput=False)
    cosT = nc.declare_dram_parameter("cosT", [HALF_ROPE, S], F32, isOutput=False)
    sinT = nc.declare_dram_parameter("sinT", [HALF_ROPE, S], F32, isOutput=False)
    o128d = nc.declare_dram_parameter("ones128", [128, 1], F32R, isOutput=False)
    o128bd = nc.declare_dram_parameter("ones128b", [128, 1], BF16, isOutput=False)
    lngd = nc.declare_dram_parameter("lng", [1, LH], F32, isOutput=False)
    out = nc.declare_dram_parameter("out", [S, D], F32, isOutput=True)

    with TileContext(nc) as tc:
        with (
            tc.tile_pool(name="cp", bufs=1) as cp,
            tc.tile_pool(name="xap", bufs=16) as xap,
            tc.tile_pool(name="wqp", bufs=6) as wqp,
            tc.tile_pool(name="wpp", bufs=9) as wpp,
            tc.tile_pool(name="sqp", bufs=2) as sqp,
            tc.tile_pool(name="exq", bufs=4) as exq,
            tc.tile_pool(name="rsp", bufs=3) as rsp,
            tc.tile_pool(name="rows", bufs=4) as rows,
            tc.tile_pool(name="stgp", bufs=3) as stgp,
            tc.tile_pool(name="tmpp", bufs=2) as tmpp,
            tc.tile_pool(name="qnp", bufs=1) as qnp,
            tc.tile_pool(name="pu", bufs=1, space="PSUM") as pu,
        ):
            wk_t = cp.tile([128, NDC, LKV * HD], F32R, tag="wk")
            nc.sync.dma_start(out=wk_t[:],
                              in_=wkT.rearrange("(c p) e -> p c e", p=128))
            wv_t = cp.tile([128, NDC, LKV * HD], F32R, tag="wv")
            nc.sync.dma_start(out=wv_t[:],
                              in_=wvT.rearrange("(c p) e -> p c e", p=128))
            cos_t = cp.tile([HALF_ROPE, S], F32, tag="cos")
            nc.sync.dma_start(out=cos_t[:], in_=cosT[:])
            sin_t = cp.tile([HALF_ROPE, S], F32, tag="sin")
            nc.sync.dma_start(out=sin_t[:], in_=sinT[:])
            o128 = cp.tile([128, 1], F32R, tag="o128")
            nc.sync.dma_start(out=o128[:], in_=o128d[:])
            o128b = cp.tile([128, 1], BF16, tag="o128b")
            nc.sync.dma_start(out=o128b[:], in_=o128bd[:])
            lng = cp.tile([1, LH], F32, tag="lng")
            nc.sync.dma_start(out=lng[:], in_=lngd[:])
            kn_t = cp.tile([128, LKV, S], F32R, tag="kn")
            v_t = cp.tile([128, S // 128, LKV * HD], BF16, tag="v")
            rkT = cp.tile([128, LKV, S // 128], F32, tag="rkT")

            def bank(tag, shape=(128, 512), dt=F32, nm=None):
                return pu.tile(list(shape), dt, tag=tag, name=nm or tag)

            def load_x_slices(pos0):
                """x chunk as 16 per-dc [128,512] pool tiles."""
                xs = []
                for dc in range(NDC):
                    xa = xap.tile([128, 512], F32R, tag="xa", name="xa")
                    nc.sync.dma_start(
                        out=xa[:],
                        in_=xT[dc * 128:(dc + 1) * 128, pos0:pos0 + 512])
                    xs.append(xa)
                return xs

            def rope_into(dst, raw, pos0):
                """dst[0:64] = rotate(raw[0:64]) with cos/sin; dst[64:128]
                copied from raw. raw is PSUM, dst is SBUF."""
                cs = slice(pos0, pos0 + 512)
                h1, h2 = slice(0, HALF_ROPE), slice(HALF_ROPE, ROPE)
                tmp = tmpp.tile([ROPE, 512], F32R, tag="tmp", name="tmp")
                nc.vector.tensor_mul(dst[h1, :], raw[h1, :], cos_t[:, cs])
                nc.vector.tensor_mul(tmp[h1, :], raw[h2, :], sin_t[:, cs])
                nc.vector.tensor_add(dst[h1, :], dst[h1, :], tmp[h1, :])
                nc.vector.tensor_mul(dst[h2, :], raw[h2, :], cos_t[:, cs])
                nc.vector.tensor_mul(tmp[h2, :], raw[h1, :], sin_t[:, cs])
                nc.vector.tensor_sub(dst[h2, :], dst[h2, :], tmp[h2, :])
                nc.scalar.copy(dst[ROPE:128, :], raw[ROPE:128, :])

            # ---------------- Phase A: kT (roped, unnormalized), rkT, v ----
            for sc in range(NQC):
                xs = load_x_slices(sc * 512)
                for g in range(LKV):
                    kacc = bank(f"b{6 + g}", nm=f"kacc{g}")
                    for dc in range(NDC):
                        nc.tensor.matmul(
                            kacc[:], wk_t[:, dc, g * HD:(g + 1) * HD],
                            xs[dc], start=(dc == 0), stop=(dc == NDC - 1))
                    # rk chunk: per-key rsqrt(mean(k^2)+eps), transposed.
                    # bf16 operands: fp32r disallows N=1 moving dims.
                    sq = sqp.tile([128, 512], BF16, tag="sqk", name="sqk")
                    nc.scalar.activation(sq[:], kacc[:], AF.Square)
                    ssqT = bank(f"b{4 + g}", (128, 4), nm=f"ssqT{g}")
                    for j in range(4):
                        nc.tensor.matmul(ssqT[:, j:j + 1],
                                         sq[:, j * 128:(j + 1) * 128],
                                         o128b[:], start=True, stop=True)
                    lr = rows.tile([128, 4], F32, tag="lr", name="lr")
                    nc.scalar.activation(lr[:], ssqT[:], AF.Ln,
                                         scale=1.0 / HD, bias=EPS)
                    nc.scalar.activation(
                        rkT[:, g, sc * 4:(sc + 1) * 4], lr[:], AF.Exp,
                        scale=-0.5)
                    rope_into(kn_t[:, g, sc * 512:(sc + 1) * 512], kacc,
                              sc * 512)
                for st4 in range(4):
                    st = sc * 4 + st4
                    vacc = bank(f"b{4 + st4 % 2}", (128, LKV * HD),
                                nm=f"vacc{st4}")
                    for dc in range(NDC):
                        nc.tensor.matmul(
                            vacc[:], xs[dc][:, st4 * 128:(st4 + 1) * 128],
                            wv_t[:, dc], start=(dc == 0), stop=(dc == NDC - 1))
                    nc.vector.tensor_copy(v_t[:, st], vacc[:])

            # ------------- Phase C: per query chunk q/attn/proj -------------
            for qc in range(NQC):
                pos0 = qc * 512
                n_kt = (qc + 1) * 4

                # -- q projection (four groups of 2 heads) + norm + rope --
                xs = load_x_slices(pos0)
                qn = {}
                for grp in range(4):
                    qraw = {}
                    for dc in range(NDC):
                        wqt = wqp.tile([128, 256], F32R, tag="wq", name="wq")
                        nc.sync.dma_start(
                            out=wqt[:],
                            in_=wqT[dc * 128:(dc + 1) * 128,
                                    grp * 256:(grp + 1) * 256])
                        for hh in range(2):
                            if dc == 0:
                                qraw[hh] = bank(f"b{(2 * grp + hh) % 4}",
                                                nm=f"qraw{grp}_{hh}")
                            nc.tensor.matmul(
                                qraw[hh][:], wqt[:, hh * HD:(hh + 1) * HD],
                                xs[dc], start=(dc == 0),
                                stop=(dc == NDC - 1))
                    for hh in range(2):
                        h = grp * 2 + hh
                        # 1/rms row with gain*scale folded in (ln/exp)
                        sq = sqp.tile([128, 512], F32R, tag="sq", name="sqq")
                        nc.scalar.activation(sq[:], qraw[hh][:], AF.Square)
                        ssq = bank("b7", (1, 512), nm=f"ssq{h}")
                        nc.tensor.matmul(ssq[:], o128[:], sq[:],
                                         start=True, stop=True)
                        tl = rows.tile([1, 512], F32, tag="tl", name="tl")
                        nc.scalar.activation(tl[:], ssq[:], AF.Ln,
                                             scale=1.0 / HD, bias=EPS)
                        r = rows.tile([1, 512], F32, tag="rr", name="rr")
                        nc.scalar.activation(r[:], tl[:], AF.Exp,
                                             scale=-0.5,
                                             bias=lng[0:1, h:h + 1])
                        rsb = rsp.tile([128, 512], F32, tag="rsb", name="rsb")
                        nc.gpsimd.partition_broadcast(rsb[:], r[:])
                        qn[h] = qnp.tile([128, 512], F32R, tag=f"qn{h}",
                                         name=f"qn{h}")
                        rope_into(qn[h][:], qraw[hh], pos0)
                        nc.vector.tensor_mul(qn[h][:], qn[h][:], rsb[:])

                # -- attention --
                yt_sb = {}
                for h in range(LH):
                    g = h // 4
                    yt_ps = bank(f"b{4 + h % 2}", nm=f"yt{h}")
                    l_ps = bank("b6", (1, 512), nm=f"l{h}")
                    for kt in range(n_kt):
                        sc_ps = bank(f"b{kt % 4}", nm=f"sc{h}_{kt}")
                        nc.tensor.matmul(
                            sc_ps[:], kn_t[:, g, kt * 128:(kt + 1) * 128],
                            qn[h][:], start=True, stop=True)
                        ex = exq.tile([128, 512], BF16, tag="ex", name="ex")
                        nc.scalar.activation(ex[:], sc_ps[:], AF.Exp,
                                             scale=rkT[:, g, kt:kt + 1])
                        m = kt - qc * 4
                        if m >= 0:
                            # zero ex where key pos > query pos:
                            # keep iff n - p - 128*m >= 0
                            nc.gpsimd.affine_select(
                                out=ex[:], in_=ex[:],
                                compare_op=mybir.AluOpType.is_ge,
                                fill=0.0, base=-128 * m,
                                pattern=[[1, 512]], channel_multiplier=-1)
                        nc.tensor.matmul(
                            yt_ps[:], v_t[:, kt, g * HD:(g + 1) * HD], ex[:],
                            start=(kt == 0), stop=(kt == n_kt - 1))
                        nc.tensor.matmul(
                            l_ps[:], o128b[:], ex[:], start=(kt == 0),
                            stop=(kt == n_kt - 1))
                    # 1/l via ln/exp + partition broadcast
                    tl = rows.tile([1, 512], F32, tag="tl", name=f"tli{h}")
                    nc.scalar.activation(tl[:], l_ps[:], AF.Ln)
                    linv = rows.tile([1, 512], F32, tag="li", name=f"li{h}")
                    nc.scalar.activation(linv[:], tl[:], AF.Exp, scale=-1.0)
                    lsb = rsp.tile([128, 512], F32, tag="rsb", name=f"lsb{h}")
                    nc.gpsimd.partition_broadcast(lsb[:], linv[:])
                    yt_sb[h] = qnp.tile([128, 512], BF16, tag=f"yts{h}",
                                        name=f"yts{h}")
                    nc.vector.tensor_mul(yt_sb[h][:], yt_ps[:], lsb[:])

                # -- output projection: out[s_q, j] += yT.T @ wpB --
                # st4-outer: one 8-matmul chain per (jcol, st4) -> 1 bank each
                for jcol in range(4):
                    wpt = {}
                    for h in range(LH):
                        wpt[h] = wpp.tile([128, 512], BF16, tag="wp",
                                          name="wp")
                        nc.sync.dma_start(
                            out=wpt[h][:],
                            in_=wpB[h * 128:(h + 1) * 128,
                                    jcol * 512:(jcol + 1) * 512])
                    for st4 in range(4):
                        prs = bank(f"b{(jcol * 4 + st4) % 4}",
                                   nm=f"pr{jcol}{st4}")
                        for h in range(LH):
                            nc.tensor.matmul(
                                prs[:],
                                yt_sb[h][:, st4 * 128:(st4 + 1) * 128],
                                wpt[h][:], start=(h == 0), stop=(h == LH - 1))
                        stg = stgp.tile([128, 512], F32, tag="stg", name="stg")
                        nc.vector.tensor_copy(stg[:], prs[:])
                        nc.sync.dma_start(
                            out=out[pos0 + st4 * 128:pos0 + (st4 + 1) * 128,
                                    jcol * 512:(jcol + 1) * 512],
                            in_=stg[:])
    nc.compile()
    return nc


def _rope_tables():
    inv = 1.0 / (10000.0 ** (np.arange(0, ROPE, 2, dtype=np.float64) / ROPE))
    fr = np.outer(np.arange(S, dtype=np.float64), inv)  # [S, 32]
    return (np.cos(fr).T.astype(np.float32).copy(),
            np.sin(fr).T.astype(np.float32).copy())


def kernel(x, Wq, Wk, Wv, Wproj, q_gain):
    global _cached_program, _last_in_maps
    x = np.ascontiguousarray(np.asarray(x, dtype=np.float32))
    Wq = np.asarray(Wq, dtype=np.float32)
    Wk = np.asarray(Wk, dtype=np.float32)
    Wv = np.asarray(Wv, dtype=np.float32)
    Wproj = np.asarray(Wproj, dtype=np.float32)
    q_gain = np.asarray(q_gain, dtype=np.float32)

    cosT, sinT = _rope_tables()
    ones128 = np.ones((128, 1), dtype=np.float32)
    ones128b = np.ones((128, 1), dtype=ml_dtypes.bfloat16)
    scale = 1.0 / np.sqrt(HD)

    in_maps = []
    for core in range(N_CORES):
        b, half = core // 2, core % 2
        g0 = half * LKV
        lng = np.log(q_gain[half * LH:(half + 1) * LH] * scale)[None, :] \
            .astype(np.float32)
        in_maps.append({
            "xT": np.ascontiguousarray(x[b].T),
            "wqT": np.ascontiguousarray(
                Wq[half * LH * HD:(half + 1) * LH * HD, :].T),
            "wkT": np.ascontiguousarray(
                Wk[g0 * HD:(g0 + LKV) * HD, :].T),
            "wvT": np.ascontiguousarray(
                Wv[g0 * HD:(g0 + LKV) * HD, :].T),
            "wpB": np.ascontiguousarray(
                Wproj[:, half * LH * HD:(half + 1) * LH * HD].T
            ).astype(ml_dtypes.bfloat16),
            "cosT": cosT, "sinT": sinT,
            "ones128": ones128, "ones128b": ones128b, "lng": lng,
        })

    _last_in_maps = in_maps
    if _cached_program is None:
        _cached_program = _build_program()
    res = run_bass_kernel_spmd(_cached_program, in_maps, list(range(N_CORES)))

    out = np.empty((B, S, D), dtype=np.float32)
    for b in range(B):
        out[b] = res.results[2 * b]["out"] + res.results[2 * b + 1]["out"]
    return out
